# revision 26
# baseline (speedup 1.0000x reference)
"""Trainium2 Bass kernel for nn_BinarizeLayer (Otsu grayscale binarization).

One NEFF, SPMD on 8 NeuronCores, data-parallel over batch (1 image/core):
  1. Stream the image in (16 chunks), fuse RGB->gray on ACT+DVE, track local
     min/max per chunk; gray [128, 12288] f32 stays resident in SBUF.
  2. Partition-transpose local min/max through DRAM, then one
     AllReduce(max) of (-min, max) -> global vmin/vmax/span.
  3. q = floor((gray - vmin) * (1/span) * 256) as bf16 integers (no fp divide
     exists on this stack; reciprocal is bitwise IEEE 1/x and, verified on the
     real data, moves zero pixels across bins vs true division; the floor's
     int32 round-trip carries an is_gt fixup so any convert rounding works).
  4. 256 global-histogram passes split across two engines in parallel:
       ACT  bin b (141 bins): accum = sum(Sign(q - b + 0.5))  [never 0 on
            integer q, so counts are exact: cnt_ge = (accum + ppb)/2]
       DVE  bin b (115 bins): fused tensor_scalar(is_ge, add, accum_out)
     Both run at 1 elem/lane/cycle - the fused-accumulate ISA has no faster
     mode - so the split is the throughput floor: ~1.49 ms of the ~1.8 ms
     kernel. Per-partition partials reduce over partitions via a stationary
     ones-matmul on the PE into PSUM, then AllReduce(add) of 256 floats.
  5. Otsu on [1, 256]: cumsums via tensor_tensor_scan (reverse via negative-
     stride APs), means via reciprocal+mul, first-argmax via masked min-
     reduce, threshold t = vmin + ((argmax + 0.5) * span) / 256 in the
     reference's fp32 op order; broadcast via a K=1 ones-matmul.
  6. Binarize gray > t, replicate x3 channels (DVE + 2 ACT strided copies),
     stream out.

Measured on trn2 vs the CPU-jax reference: 0/37,748,736 mismatched output
elements; HW exec ~1.80 ms (memory floor ~0.2 ms, histogram wall dominates).
"""

import sys

sys.path.insert(0, "/opt/trn_rl_repo")

from contextlib import ExitStack

import numpy as np

import concourse.bass as bass
import concourse.tile as tile
from concourse import bacc, mybir
from concourse.bass_utils import run_bass_kernel_spmd

AF = mybir.AluOpType
F32 = mybir.dt.float32
X = mybir.AxisListType.X

NCORES = 8
P = 128
NB = 256
W_R, W_G, W_B = 0.2989, 0.5870, 0.1140

# Full problem: batch 8 of [1024, 1536, 3] f32; one image per core.
FULL_PPB = 12288  # pixels per partition = 1024*1536/128
FULL_NCHUNK = 16


def _kernel_body(ctx, tc, x, binc, y, ppb, nchunk, dbg=None, hist_mode="hinge", act_frac=0.5515):
    nc = tc.nc
    cpx = ppb // nchunk
    cores = list(range(NCORES))

    sb = ctx.enter_context(tc.tile_pool(name="sb", bufs=1))
    ps = ctx.enter_context(tc.tile_pool(name="ps", bufs=1, space="PSUM"))
    dram = ctx.enter_context(tc.tile_pool(name="dram", bufs=1, space="DRAM"))

    gray = sb.tile([P, ppb], F32)
    mnl = sb.tile([P, nchunk], F32)
    mxl = sb.tile([P, nchunk], F32)

    # ---- Phase 1: load, grayscale, local min/max ----
    with tc.tile_pool(name="inp", bufs=2) as inp, tc.tile_pool(name="tmp", bufs=2) as tmp:
        for c in range(nchunk):
            t_in = inp.tile([P, 3 * cpx], F32, tag="in")
            nc.sync.dma_start(t_in[:], x[:, 3 * cpx * c : 3 * cpx * (c + 1)])
            R = t_in[:, 0 : 3 * cpx : 3]
            G = t_in[:, 1 : 3 * cpx : 3]
            B = t_in[:, 2 : 3 * cpx : 3]
            gc = gray[:, cpx * c : cpx * (c + 1)]
            t1 = tmp.tile([P, cpx], F32, tag="t1")
            nc.scalar.mul(t1[:], R, W_R)
            t2 = tmp.tile([P, cpx], F32, tag="t2")
            nc.vector.scalar_tensor_tensor(t2[:], G, W_G, t1[:], op0=AF.mult, op1=AF.add)
            nc.vector.scalar_tensor_tensor(gc, B, W_B, t2[:], op0=AF.mult, op1=AF.add)
            nc.vector.tensor_reduce(mnl[:, c : c + 1], gc, axis=X, op=AF.min)
            nc.vector.tensor_reduce(mxl[:, c : c + 1], gc, axis=X, op=AF.max)

    # ---- Phase 2: global min/max via DRAM transpose + AllReduce(max) ----
    mn1 = sb.tile([P, 1], F32)
    mx1 = sb.tile([P, 1], F32)
    nc.vector.tensor_reduce(mn1[:], mnl[:], axis=X, op=AF.min)
    nc.vector.tensor_reduce(mx1[:], mxl[:], axis=X, op=AF.max)
    pk = sb.tile([P, 2], F32)
    nc.scalar.mul(pk[:, 0:1], mn1[:], -1.0)  # negate min so one max-allreduce works
    nc.scalar.copy(pk[:, 1:2], mx1[:])
    d_pk = dram.tile([1, 2 * P], F32)
    nc.sync.dma_start(d_pk[:].rearrange("o (p k) -> (o p) k", k=2), pk[:])
    row = sb.tile([1, 2 * P], F32)
    nc.sync.dma_start(row[:], d_pk[:])
    cc_in = sb.tile([1, 2], F32)
    nc.vector.tensor_reduce(cc_in[:, 0:1], row[:, 0 : 2 * P : 2], axis=X, op=AF.max)
    nc.vector.tensor_reduce(cc_in[:, 1:2], row[:, 1 : 2 * P : 2], axis=X, op=AF.max)
    d_ci = dram.tile([1, 2], F32)
    d_co = dram.tile([1, 2], F32)
    nc.sync.dma_start(d_ci[:], cc_in[:])
    nc.gpsimd.collective_compute(
        "AllReduce", AF.max, replica_groups=[cores], ins=[d_ci[:].opt()], outs=[d_co[:].opt()]
    )
    cc_out = sb.tile([1, 2], F32)
    nc.sync.dma_start(cc_out[:], d_co[:])

    vmin_t = sb.tile([1, 1], F32)
    nc.scalar.mul(vmin_t[:], cc_out[:, 0:1], -1.0)
    vmax_t = cc_out[:, 1:2]
    # span = vmax > vmin ? vmax - vmin : 1.0
    spanr = sb.tile([1, 1], F32)
    nc.vector.tensor_tensor(spanr[:], vmax_t, vmin_t[:], op=AF.subtract)
    msk = sb.tile([1, 1], F32)
    nc.vector.tensor_tensor(msk[:], vmax_t, vmin_t[:], op=AF.is_gt)
    sp_a = sb.tile([1, 1], F32)
    nc.vector.tensor_tensor(sp_a[:], spanr[:], msk[:], op=AF.mult)
    # span = spanr*msk + (1 - msk); each step exact since msk in {0, 1}
    nmsk = sb.tile([1, 1], F32)
    nc.vector.tensor_scalar(nmsk[:], msk[:], -1.0, None, op0=AF.mult)
    nc.vector.tensor_scalar(nmsk[:], nmsk[:], 1.0, None, op0=AF.add)
    span_t = sb.tile([1, 1], F32)
    nc.vector.tensor_tensor(span_t[:], sp_a[:], nmsk[:], op=AF.add)

    # ---- Broadcast vmin/rspan to all partitions via K=1 ones-matmul ----
    # No fp divide exists on this stack; use reciprocal + multiply. Verified
    # on the real data: zero pixels change bins vs IEEE division, and the
    # Otsu argmax has ~1e-4 relative headroom over count perturbations.
    rspan_t = sb.tile([1, 1], F32)
    nc.vector.reciprocal(rspan_t[:], span_t[:])
    ones_1xP = sb.tile([1, P], F32)
    nc.vector.memset(ones_1xP[:], 1.0)
    vs_in = sb.tile([1, 2], F32)
    nc.scalar.copy(vs_in[:, 0:1], vmin_t[:])
    nc.scalar.copy(vs_in[:, 1:2], rspan_t[:])
    ps_b = ps.tile([P, 2], F32)
    nc.tensor.matmul(ps_b[:], lhsT=ones_1xP[:], rhs=vs_in[:], start=True, stop=True)
    vsB = sb.tile([P, 2], F32)
    nc.vector.tensor_copy(vsB[:], ps_b[:])

    BF16 = mybir.dt.bfloat16
    I32 = mybir.dt.int32
    if hist_mode == "isge":
        # ---- v1: r per pixel; cnt_ge partials, one fused DVE op per bin ----
        r = sb.tile([P, ppb], F32)
        nc.vector.tensor_scalar(r[:], gray[:], vsB[:, 0:1], None, op0=AF.subtract)
        nc.vector.tensor_scalar(r[:], r[:], vsB[:, 1:2], None, op0=AF.mult)
        pt = sb.tile([P, NB], F32)
        trash = sb.tile([P, ppb], BF16)
        for b in range(NB):
            nc.vector.tensor_scalar(
                trash[:], r[:], float(b) / 256.0, None, op0=AF.is_ge, op1=AF.add,
                accum_out=pt[:, b : b + 1],
            )
        onesP = sb.tile([P, 1], F32)
        nc.vector.memset(onesP[:], 1.0)
        ps_cnt = ps.tile([1, NB], F32)
        nc.tensor.matmul(ps_cnt[:], lhsT=onesP[:], rhs=pt[:], start=True, stop=True)
        cnt = sb.tile([1, NB], F32)
        nc.vector.tensor_copy(cnt[:], ps_cnt[:])
        d_hi = dram.tile([1, NB], F32)
        d_ho = dram.tile([1, NB], F32)
        nc.sync.dma_start(d_hi[:], cnt[:])
        nc.gpsimd.collective_compute(
            "AllReduce", AF.add, replica_groups=[cores],
            ins=[d_hi[:].opt()], outs=[d_ho[:].opt()],
        )
        gcnt = sb.tile([1, NB], F32)
        nc.sync.dma_start(gcnt[:], d_ho[:])
        hist = sb.tile([1, NB], F32)
        nc.vector.tensor_tensor(
            hist[:, 0 : NB - 1], gcnt[:, 0 : NB - 1], gcnt[:, 1:NB], op=AF.subtract
        )
        nc.scalar.copy(hist[:, NB - 1 : NB], gcnt[:, NB - 1 : NB])
    else:
        # ---- v2 "hinge": q = clip(floor((gray-vmin)*rspan*256), 255) as bf16;
        # g(c) = sum|q-c| per partition for 258 integer edges c=-1..256, split
        # across ACT (activation Abs) and DVE (STT subtract/abs_max), then
        # hist[b] = (g(b-1) - 2 g(b) + g(b+1)) / 2 exactly. ----
        # w256 = rspan * 256 (exact power-of-2 scale; folding is bit-exact)
        w256B = sb.tile([P, 1], F32)
        nc.vector.tensor_scalar(w256B[:], vsB[:, 1:2], 256.0, None, op0=AF.mult)
        q = sb.tile([P, ppb], BF16)
        with tc.tile_pool(name="qtmp", bufs=2) as qtmp:
            qch = max(1, ppb // 8)
            for c0 in range(0, ppb, qch):
                # y = (gray - vmin) * (rspan*256); floor via int32 round-trip
                # with an is_gt fixup (correct for truncate or RNE converts).
                # No top clip: bin 255 uses cnt_ge[255], so q=256 still counts.
                yc = qtmp.tile([P, qch], F32, tag="yc")
                nc.vector.tensor_scalar(
                    yc[:], gray[:, c0 : c0 + qch], vsB[:, 0:1], w256B[:],
                    op0=AF.subtract, op1=AF.mult,
                )
                qi = qtmp.tile([P, qch], I32, tag="qi")
                nc.vector.tensor_copy(qi[:], yc[:])
                qf = qtmp.tile([P, qch], F32, tag="qf")
                nc.vector.tensor_copy(qf[:], qi[:])
                fx = qtmp.tile([P, qch], F32, tag="fx")
                nc.vector.tensor_tensor(fx[:], qf[:], yc[:], op=AF.is_gt)
                nc.vector.scalar_tensor_tensor(
                    q[:, c0 : c0 + qch], fx[:], -1.0, qf[:], op0=AF.mult, op1=AF.add
                )
        # Split the 256 cnt_ge passes across ACT and DVE:
        #   DVE bin b: accum = sum(q >= b)                  (is_ge, add)
        #   ACT bin b: accum = sum(sign(q - b + 0.5))       (never 0 on int q)
        #              -> cnt_ge = (accum + ppb) / 2 per partition, exact.
        n_act = int(NB * act_frac)
        n_dve = NB - n_act
        pt_a = sb.tile([P, max(n_act, 1)], F32)
        pt_d = sb.tile([P, max(n_dve, 1)], F32)
        trash_a = sb.tile([P, ppb], BF16)
        trash_d = sb.tile([P, ppb], BF16)
        # sbias col j (ACT bin b=j): 0.5 - b
        sbias_i = sb.tile([P, max(n_act, 1)], I32)
        nc.gpsimd.iota(sbias_i[:], pattern=[[-1, max(n_act, 1)]], base=0, channel_multiplier=0)
        sbias = sb.tile([P, max(n_act, 1)], F32)
        nc.vector.tensor_copy(sbias[:], sbias_i[:])
        nc.vector.tensor_scalar(sbias[:], sbias[:], 0.5, None, op0=AF.add)
        for j in range(n_act):
            nc.scalar.activation(
                trash_a[:], q[:], mybir.ActivationFunctionType.Sign,
                bias=sbias[:, j : j + 1], scale=1.0,
                accum_out=pt_a[:, j : j + 1],
            )
        for j in range(n_dve):
            b = float(n_act + j)
            nc.vector.tensor_scalar(
                trash_d[:], q[:], b, None, op0=AF.is_ge, op1=AF.add,
                accum_out=pt_d[:, j : j + 1],
            )
        # normalize ACT sign-sums to cnt_ge: (s + ppb) * 0.5, exact
        if n_act:
            nc.vector.tensor_scalar(
                pt_a[:], pt_a[:], float(ppb), 0.5, op0=AF.add, op1=AF.mult
            )
        onesP = sb.tile([P, 1], F32)
        nc.vector.memset(onesP[:], 1.0)
        ps_cnt = ps.tile([1, NB], F32)
        if n_act:
            nc.tensor.matmul(ps_cnt[:, 0:n_act], lhsT=onesP[:], rhs=pt_a[:], start=True, stop=True)
        if n_dve:
            nc.tensor.matmul(ps_cnt[:, n_act:NB], lhsT=onesP[:], rhs=pt_d[:], start=True, stop=True)
        cnt = sb.tile([1, NB], F32)
        nc.vector.tensor_copy(cnt[:], ps_cnt[:])
        d_hi = dram.tile([1, NB], F32)
        d_ho = dram.tile([1, NB], F32)
        nc.sync.dma_start(d_hi[:], cnt[:])
        nc.gpsimd.collective_compute(
            "AllReduce", AF.add, replica_groups=[cores],
            ins=[d_hi[:].opt()], outs=[d_ho[:].opt()],
        )
        gcnt = sb.tile([1, NB], F32)
        nc.sync.dma_start(gcnt[:], d_ho[:])
        hist = sb.tile([1, NB], F32)
        nc.vector.tensor_tensor(
            hist[:, 0 : NB - 1], gcnt[:, 0 : NB - 1], gcnt[:, 1:NB], op=AF.subtract
        )
        nc.scalar.copy(hist[:, NB - 1 : NB], gcnt[:, NB - 1 : NB])

    # ---- Phase 6: Otsu on [1, 256] ----
    binc_s = sb.tile([1, NB], F32)
    nc.sync.dma_start(binc_s[:], binc[:])  # arange(256) + 0.5
    centers = sb.tile([1, NB], F32)
    nc.vector.tensor_scalar(centers[:], binc_s[:], span_t[:], None, op0=AF.mult)
    nc.vector.tensor_scalar(centers[:], centers[:], 1.0 / 256.0, None, op0=AF.mult)
    nc.vector.tensor_scalar(centers[:], centers[:], vmin_t[:], None, op0=AF.add)
    hc = sb.tile([1, NB], F32)
    nc.vector.tensor_tensor(hc[:], hist[:], centers[:], op=AF.mult)

    z256 = sb.tile([1, NB], F32)
    nc.vector.memset(z256[:], 0.0)
    w1 = sb.tile([1, NB], F32)
    nc.vector.tensor_tensor_scan(w1[:], hist[:], z256[:], 0.0, op0=AF.add, op1=AF.add)
    c1 = sb.tile([1, NB], F32)
    nc.vector.tensor_tensor_scan(c1[:], hc[:], z256[:], 0.0, op0=AF.add, op1=AF.add)
    w2 = sb.tile([1, NB], F32)
    nc.vector.tensor_tensor_scan(
        w2[:, ::-1], hist[:, ::-1], z256[:], 0.0, op0=AF.add, op1=AF.add
    )
    c2 = sb.tile([1, NB], F32)
    nc.vector.tensor_tensor_scan(
        c2[:, ::-1], hc[:, ::-1], z256[:], 0.0, op0=AF.add, op1=AF.add
    )
    d1 = sb.tile([1, NB], F32)
    nc.vector.tensor_scalar(d1[:], w1[:], 1e-12, None, op0=AF.max)
    nc.vector.reciprocal(d1[:], d1[:])
    m1 = sb.tile([1, NB], F32)
    nc.vector.tensor_tensor(m1[:], c1[:], d1[:], op=AF.mult)
    d2 = sb.tile([1, NB], F32)
    nc.vector.tensor_scalar(d2[:], w2[:], 1e-12, None, op0=AF.max)
    nc.vector.reciprocal(d2[:], d2[:])
    m2 = sb.tile([1, NB], F32)
    nc.vector.tensor_tensor(m2[:], c2[:], d2[:], op=AF.mult)

    nv = NB - 1
    dd = sb.tile([1, nv], F32)
    nc.vector.tensor_tensor(dd[:], m1[:, 0:nv], m2[:, 1:NB], op=AF.subtract)
    ddsq = sb.tile([1, nv], F32)
    nc.vector.tensor_tensor(ddsq[:], dd[:], dd[:], op=AF.mult)
    vv = sb.tile([1, nv], F32)
    nc.vector.tensor_tensor(vv[:], w1[:, 0:nv], w2[:, 1:NB], op=AF.mult)
    var12 = sb.tile([1, nv], F32)
    nc.vector.tensor_tensor(var12[:], vv[:], ddsq[:], op=AF.mult)

    vmx = sb.tile([1, 1], F32)
    nc.vector.tensor_reduce(vmx[:], var12[:], axis=X, op=AF.max)
    eqm = sb.tile([1, nv], F32)
    nc.vector.tensor_scalar(eqm[:], var12[:], vmx[:], None, op0=AF.is_equal)
    BIG = 1.0e9
    # cand = (1 - eqm)*BIG + binc: exact binc (= idx + 0.5) at max positions,
    # ~BIG elsewhere. (1-eqm) computed exactly first to avoid cancellation.
    neq = sb.tile([1, nv], F32)
    nc.vector.tensor_scalar(neq[:], eqm[:], -1.0, None, op0=AF.mult)
    nc.vector.tensor_scalar(neq[:], neq[:], 1.0, None, op0=AF.add)
    cand = sb.tile([1, nv], F32)
    nc.vector.scalar_tensor_tensor(
        cand[:], neq[:], BIG, binc_s[:, 0:nv], op0=AF.mult, op1=AF.add
    )
    idxf = sb.tile([1, 1], F32)
    nc.vector.tensor_reduce(idxf[:], cand[:], axis=X, op=AF.min)
    # t = vmin + ((idx + 0.5) * span) / 256 ; idxf = idx + 0.5 already
    tt = sb.tile([1, 1], F32)
    nc.vector.tensor_scalar(tt[:], idxf[:], span_t[:], None, op0=AF.mult)
    nc.vector.tensor_scalar(tt[:], tt[:], 1.0 / 256.0, None, op0=AF.mult)
    nc.vector.tensor_scalar(tt[:], tt[:], vmin_t[:], None, op0=AF.add)

    # broadcast threshold
    ps_t = ps.tile([P, 1], F32)
    nc.tensor.matmul(ps_t[:], lhsT=ones_1xP[:], rhs=tt[:], start=True, stop=True)
    tB = sb.tile([P, 1], F32)
    nc.vector.tensor_copy(tB[:], ps_t[:])

    if dbg is not None:
        dtile = sb.tile([1, 1024], F32)
        nc.vector.memset(dtile[:], 0.0)
        nc.scalar.copy(dtile[:, 0:1], vmin_t[:])
        nc.scalar.copy(dtile[:, 1:2], span_t[:])
        nc.scalar.copy(dtile[:, 2:3], tt[:])
        nc.scalar.copy(dtile[:, 3:4], idxf[:])
        nc.scalar.copy(dtile[:, 4:5], vmx[:])
        nc.vector.tensor_copy(dtile[:, 256:512], gcnt[:])
        nc.vector.tensor_copy(dtile[:, 512:768], hist[:])
        nc.vector.tensor_copy(dtile[:, 768:1023], var12[:])
        nc.sync.dma_start(dbg[:], dtile[:])

    # ---- Phase 7: binarize + replicate x3 + store ----
    # Replication copies split DVE/ACT so both engines drain the tail.
    with tc.tile_pool(name="outp", bufs=3) as outp:
        for c in range(nchunk):
            gc = gray[:, cpx * c : cpx * (c + 1)]
            b01 = outp.tile([P, cpx], F32, tag="b01")
            nc.vector.tensor_scalar(b01[:], gc, tB[:], None, op0=AF.is_gt)
            o3 = outp.tile([P, 3 * cpx], F32, tag="o3")
            nc.vector.tensor_copy(o3[:, 0 : 3 * cpx : 3], b01[:])
            nc.scalar.copy(o3[:, 1 : 3 * cpx : 3], b01[:])
            nc.scalar.copy(o3[:, 2 : 3 * cpx : 3], b01[:])
            nc.sync.dma_start(y[:, 3 * cpx * c : 3 * cpx * (c + 1)], o3[:])


def build_nc(ppb=FULL_PPB, nchunk=FULL_NCHUNK, debug=False, enable_asserts=False,
             with_dbg=False, hist_mode="hinge", act_frac=0.5515):
    nc = bacc.Bacc(
        "TRN2",
        target_bir_lowering=False,
        debug=debug,
        enable_asserts=enable_asserts,
        num_devices=NCORES,
    )
    x = nc.dram_tensor("x", [P, 3 * ppb], F32, kind="ExternalInput")
    binc = nc.dram_tensor("binc", [1, NB], F32, kind="ExternalInput")
    y = nc.dram_tensor("y", [P, 3 * ppb], F32, kind="ExternalOutput")
    dbg = (
        nc.dram_tensor("dbg", [1, 1024], F32, kind="ExternalOutput")
        if with_dbg
        else None
    )
    with tile.TileContext(nc) as tc:
        with ExitStack() as ctx:
            _kernel_body(
                ctx, tc, x.ap(), binc.ap(), y.ap(), ppb, nchunk,
                dbg=dbg.ap() if dbg is not None else None,
                hist_mode=hist_mode, act_frac=act_frac,
            )
    nc.compile()
    return nc


_NC_CACHE = {}


def _get_nc():
    key = (FULL_PPB, FULL_NCHUNK)
    if key not in _NC_CACHE:
        _NC_CACHE[key] = build_nc()
    return _NC_CACHE[key]


def make_in_maps(inputs_np):
    """inputs_np: [8, 1024, 1536, 3] f32 -> per-core in_maps."""
    binc = (np.arange(NB, dtype=np.float32) + 0.5).reshape(1, NB)
    maps = []
    for c in range(NCORES):
        img = np.ascontiguousarray(inputs_np[c]).reshape(P, 3 * FULL_PPB)
        maps.append({"x": img, "binc": binc})
    return maps


def kernel(inputs: np.ndarray) -> np.ndarray:
    inputs = np.asarray(inputs, dtype=np.float32)
    assert inputs.shape == (8, 1024, 1536, 3), inputs.shape
    nc = _get_nc()
    res = run_bass_kernel_spmd(nc, make_in_maps(inputs), list(range(NCORES)))
    out = np.stack(
        [res.results[c]["y"].reshape(1024, 1536, 3) for c in range(NCORES)], axis=0
    )
    return out


if __name__ == "__main__":
    rng = np.random.default_rng(0)
    x = rng.random((8, 1024, 1536, 3), dtype=np.float32)
    y = kernel(x)
    print(y.shape, y.dtype, y.mean())


# revision 27
# speedup vs baseline: 1.0025x; 1.0025x over previous
"""Trainium2 Bass kernel for nn_BinarizeLayer (Otsu grayscale binarization).

One NEFF, SPMD on 8 NeuronCores, data-parallel over batch (1 image/core):
  1. Stream the image in (16 chunks), fuse RGB->gray on ACT+DVE, track local
     min/max per chunk; gray [128, 12288] f32 stays resident in SBUF.
  2. Partition-transpose local min/max through DRAM, then one
     AllReduce(max) of (-min, max) -> global vmin/vmax/span.
  3. q = floor((gray - vmin) * (1/span) * 256) as bf16 integers (no fp divide
     exists on this stack; reciprocal is bitwise IEEE 1/x and, verified on the
     real data, moves zero pixels across bins vs true division; the floor's
     int32 round-trip carries an is_gt fixup so any convert rounding works).
  4. 256 global-histogram passes split across two engines in parallel:
       ACT  bin b (141 bins): accum = sum(Sign(q - b + 0.5))  [never 0 on
            integer q, so counts are exact: cnt_ge = (accum + ppb)/2]
       DVE  bin b (115 bins): fused tensor_scalar(is_ge, add, accum_out)
     Both run at 1 elem/lane/cycle - the fused-accumulate ISA has no faster
     mode - so the split is the throughput floor: ~1.49 ms of the ~1.8 ms
     kernel. Per-partition partials reduce over partitions via a stationary
     ones-matmul on the PE into PSUM, then AllReduce(add) of 256 floats.
  5. Otsu on [1, 256]: cumsums via tensor_tensor_scan (reverse via negative-
     stride APs), means via reciprocal+mul, first-argmax via masked min-
     reduce, threshold t = vmin + ((argmax + 0.5) * span) / 256 in the
     reference's fp32 op order; broadcast via a K=1 ones-matmul.
  6. Binarize gray > t, replicate x3 channels (DVE + 2 ACT strided copies),
     stream out.

Measured on trn2 vs the CPU-jax reference: 0/37,748,736 mismatched output
elements; HW exec ~1.80 ms (memory floor ~0.2 ms, histogram wall dominates).
"""

import sys

sys.path.insert(0, "/opt/trn_rl_repo")

from contextlib import ExitStack

import numpy as np

import concourse.bass as bass
import concourse.tile as tile
from concourse import bacc, mybir
from concourse.bass_utils import run_bass_kernel_spmd

AF = mybir.AluOpType
F32 = mybir.dt.float32
X = mybir.AxisListType.X

NCORES = 8
P = 128
NB = 256
W_R, W_G, W_B = 0.2989, 0.5870, 0.1140

# Full problem: batch 8 of [1024, 1536, 3] f32; one image per core.
FULL_PPB = 12288  # pixels per partition = 1024*1536/128
FULL_NCHUNK = 16


def _kernel_body(ctx, tc, x, binc, y, ppb, nchunk, dbg=None, hist_mode="hinge", act_frac=0.5515):
    nc = tc.nc
    cpx = ppb // nchunk
    cores = list(range(NCORES))

    sb = ctx.enter_context(tc.tile_pool(name="sb", bufs=1))
    ps = ctx.enter_context(tc.tile_pool(name="ps", bufs=1, space="PSUM"))
    dram = ctx.enter_context(tc.tile_pool(name="dram", bufs=1, space="DRAM"))

    gray = sb.tile([P, ppb], F32)
    mnl = sb.tile([P, nchunk], F32)
    mxl = sb.tile([P, nchunk], F32)

    # ---- Phase 1: load, grayscale, local min/max ----
    with tc.tile_pool(name="inp", bufs=2) as inp, tc.tile_pool(name="tmp", bufs=2) as tmp:
        for c in range(nchunk):
            t_in = inp.tile([P, 3 * cpx], F32, tag="in")
            nc.sync.dma_start(t_in[:], x[:, 3 * cpx * c : 3 * cpx * (c + 1)])
            R = t_in[:, 0 : 3 * cpx : 3]
            G = t_in[:, 1 : 3 * cpx : 3]
            B = t_in[:, 2 : 3 * cpx : 3]
            gc = gray[:, cpx * c : cpx * (c + 1)]
            t1 = tmp.tile([P, cpx], F32, tag="t1")
            nc.scalar.mul(t1[:], R, W_R)
            t2 = tmp.tile([P, cpx], F32, tag="t2")
            nc.vector.scalar_tensor_tensor(t2[:], G, W_G, t1[:], op0=AF.mult, op1=AF.add)
            nc.vector.scalar_tensor_tensor(gc, B, W_B, t2[:], op0=AF.mult, op1=AF.add)
            nc.vector.tensor_reduce(mnl[:, c : c + 1], gc, axis=X, op=AF.min)
            nc.vector.tensor_reduce(mxl[:, c : c + 1], gc, axis=X, op=AF.max)

    # ---- Phase 2: global min/max via DRAM transpose + AllReduce(max) ----
    mn1 = sb.tile([P, 1], F32)
    mx1 = sb.tile([P, 1], F32)
    nc.vector.tensor_reduce(mn1[:], mnl[:], axis=X, op=AF.min)
    nc.vector.tensor_reduce(mx1[:], mxl[:], axis=X, op=AF.max)
    pk = sb.tile([P, 2], F32)
    nc.scalar.mul(pk[:, 0:1], mn1[:], -1.0)  # negate min so one max-allreduce works
    nc.scalar.copy(pk[:, 1:2], mx1[:])
    d_pk = dram.tile([1, 2 * P], F32)
    nc.sync.dma_start(d_pk[:].rearrange("o (p k) -> (o p) k", k=2), pk[:])
    row = sb.tile([1, 2 * P], F32)
    nc.sync.dma_start(row[:], d_pk[:])
    cc_in = sb.tile([1, 2], F32)
    nc.vector.tensor_reduce(cc_in[:, 0:1], row[:, 0 : 2 * P : 2], axis=X, op=AF.max)
    nc.vector.tensor_reduce(cc_in[:, 1:2], row[:, 1 : 2 * P : 2], axis=X, op=AF.max)
    d_ci = dram.tile([1, 2], F32)
    d_co = dram.tile([1, 2], F32)
    nc.sync.dma_start(d_ci[:], cc_in[:])
    nc.gpsimd.collective_compute(
        "AllReduce", AF.max, replica_groups=[cores], ins=[d_ci[:].opt()], outs=[d_co[:].opt()]
    )
    cc_out = sb.tile([1, 2], F32)
    nc.sync.dma_start(cc_out[:], d_co[:])

    vmin_t = sb.tile([1, 1], F32)
    nc.scalar.mul(vmin_t[:], cc_out[:, 0:1], -1.0)
    vmax_t = cc_out[:, 1:2]
    # span = vmax > vmin ? vmax - vmin : 1.0
    spanr = sb.tile([1, 1], F32)
    nc.vector.tensor_tensor(spanr[:], vmax_t, vmin_t[:], op=AF.subtract)
    msk = sb.tile([1, 1], F32)
    nc.vector.tensor_tensor(msk[:], vmax_t, vmin_t[:], op=AF.is_gt)
    sp_a = sb.tile([1, 1], F32)
    nc.vector.tensor_tensor(sp_a[:], spanr[:], msk[:], op=AF.mult)
    # span = spanr*msk + (1 - msk); each step exact since msk in {0, 1}
    nmsk = sb.tile([1, 1], F32)
    nc.vector.tensor_scalar(nmsk[:], msk[:], -1.0, None, op0=AF.mult)
    nc.vector.tensor_scalar(nmsk[:], nmsk[:], 1.0, None, op0=AF.add)
    span_t = sb.tile([1, 1], F32)
    nc.vector.tensor_tensor(span_t[:], sp_a[:], nmsk[:], op=AF.add)

    # ---- Broadcast vmin/rspan to all partitions via K=1 ones-matmul ----
    # No fp divide exists on this stack; use reciprocal + multiply. Verified
    # on the real data: zero pixels change bins vs IEEE division, and the
    # Otsu argmax has ~1e-4 relative headroom over count perturbations.
    rspan_t = sb.tile([1, 1], F32)
    nc.vector.reciprocal(rspan_t[:], span_t[:])
    ones_1xP = sb.tile([1, P], F32)
    nc.vector.memset(ones_1xP[:], 1.0)
    vs_in = sb.tile([1, 2], F32)
    nc.scalar.copy(vs_in[:, 0:1], vmin_t[:])
    nc.scalar.copy(vs_in[:, 1:2], rspan_t[:])
    ps_b = ps.tile([P, 2], F32)
    nc.tensor.matmul(ps_b[:], lhsT=ones_1xP[:], rhs=vs_in[:], start=True, stop=True)
    vsB = sb.tile([P, 2], F32)
    nc.vector.tensor_copy(vsB[:], ps_b[:])

    BF16 = mybir.dt.bfloat16
    I32 = mybir.dt.int32
    if hist_mode == "isge":
        # ---- v1: r per pixel; cnt_ge partials, one fused DVE op per bin ----
        r = sb.tile([P, ppb], F32)
        nc.vector.tensor_scalar(r[:], gray[:], vsB[:, 0:1], None, op0=AF.subtract)
        nc.vector.tensor_scalar(r[:], r[:], vsB[:, 1:2], None, op0=AF.mult)
        pt = sb.tile([P, NB], F32)
        trash = sb.tile([P, ppb], BF16)
        for b in range(NB):
            nc.vector.tensor_scalar(
                trash[:], r[:], float(b) / 256.0, None, op0=AF.is_ge, op1=AF.add,
                accum_out=pt[:, b : b + 1],
            )
        onesP = sb.tile([P, 1], F32)
        nc.vector.memset(onesP[:], 1.0)
        ps_cnt = ps.tile([1, NB], F32)
        nc.tensor.matmul(ps_cnt[:], lhsT=onesP[:], rhs=pt[:], start=True, stop=True)
        cnt = sb.tile([1, NB], F32)
        nc.vector.tensor_copy(cnt[:], ps_cnt[:])
        d_hi = dram.tile([1, NB], F32)
        d_ho = dram.tile([1, NB], F32)
        nc.sync.dma_start(d_hi[:], cnt[:])
        nc.gpsimd.collective_compute(
            "AllReduce", AF.add, replica_groups=[cores],
            ins=[d_hi[:].opt()], outs=[d_ho[:].opt()],
        )
        gcnt = sb.tile([1, NB], F32)
        nc.sync.dma_start(gcnt[:], d_ho[:])
        hist = sb.tile([1, NB], F32)
        nc.vector.tensor_tensor(
            hist[:, 0 : NB - 1], gcnt[:, 0 : NB - 1], gcnt[:, 1:NB], op=AF.subtract
        )
        nc.scalar.copy(hist[:, NB - 1 : NB], gcnt[:, NB - 1 : NB])
    else:
        # ---- v2 "split": q = floor((gray-vmin)*rspan*256) as integer-valued
        # bf16, then 256 exact cnt_ge passes split across ACT and DVE (see
        # module docstring). ----
        # w256 = rspan * 256 (exact power-of-2 scale; folding is bit-exact)
        w256B = sb.tile([P, 1], F32)
        nc.vector.tensor_scalar(w256B[:], vsB[:, 1:2], 256.0, None, op0=AF.mult)
        q = sb.tile([P, ppb], BF16)
        with tc.tile_pool(name="qtmp", bufs=2) as qtmp:
            qch = max(1, ppb // 8)
            for c0 in range(0, ppb, qch):
                # y = (gray - vmin) * (rspan*256); floor via int32 round-trip
                # with an is_gt fixup (correct for truncate or RNE converts).
                # No top clip: bin 255 uses cnt_ge[255], so q=256 still counts.
                yc = qtmp.tile([P, qch], F32, tag="yc")
                nc.vector.tensor_scalar(
                    yc[:], gray[:, c0 : c0 + qch], vsB[:, 0:1], w256B[:],
                    op0=AF.subtract, op1=AF.mult,
                )
                qi = qtmp.tile([P, qch], I32, tag="qi")
                nc.vector.tensor_copy(qi[:], yc[:])
                qf = qtmp.tile([P, qch], F32, tag="qf")
                nc.vector.tensor_copy(qf[:], qi[:])
                fx = qtmp.tile([P, qch], F32, tag="fx")
                nc.vector.tensor_tensor(fx[:], qf[:], yc[:], op=AF.is_gt)
                nc.vector.scalar_tensor_tensor(
                    q[:, c0 : c0 + qch], fx[:], -1.0, qf[:], op0=AF.mult, op1=AF.add
                )
        # Split the 256 cnt_ge passes across ACT and DVE:
        #   DVE bin b: accum = sum(q >= b)                  (is_ge, add)
        #   ACT bin b: accum = sum(sign(q - b + 0.5))       (never 0 on int q)
        #              -> cnt_ge = (accum + ppb) / 2 per partition, exact.
        n_act = int(NB * act_frac)
        n_dve = NB - n_act
        pt_a = sb.tile([P, max(n_act, 1)], F32)
        pt_d = sb.tile([P, max(n_dve, 1)], F32)
        trash_a = sb.tile([P, ppb], BF16)
        trash_d = sb.tile([P, ppb], BF16)
        # sbias col j (ACT bin b=j): 0.5 - b
        sbias_i = sb.tile([P, max(n_act, 1)], I32)
        nc.gpsimd.iota(sbias_i[:], pattern=[[-1, max(n_act, 1)]], base=0, channel_multiplier=0)
        sbias = sb.tile([P, max(n_act, 1)], F32)
        nc.vector.tensor_copy(sbias[:], sbias_i[:])
        nc.vector.tensor_scalar(sbias[:], sbias[:], 0.5, None, op0=AF.add)
        for j in range(n_act):
            nc.scalar.activation(
                trash_a[:], q[:], mybir.ActivationFunctionType.Sign,
                bias=sbias[:, j : j + 1], scale=1.0,
                accum_out=pt_a[:, j : j + 1],
            )
        for j in range(n_dve):
            b = float(n_act + j)
            nc.vector.tensor_scalar(
                trash_d[:], q[:], b, None, op0=AF.is_ge, op1=AF.add,
                accum_out=pt_d[:, j : j + 1],
            )
        # normalize ACT sign-sums to cnt_ge: (s + ppb) * 0.5, exact
        if n_act:
            nc.vector.tensor_scalar(
                pt_a[:], pt_a[:], float(ppb), 0.5, op0=AF.add, op1=AF.mult
            )
        onesP = sb.tile([P, 1], F32)
        nc.vector.memset(onesP[:], 1.0)
        ps_cnt = ps.tile([1, NB], F32)
        if n_act:
            nc.tensor.matmul(ps_cnt[:, 0:n_act], lhsT=onesP[:], rhs=pt_a[:], start=True, stop=True)
        if n_dve:
            nc.tensor.matmul(ps_cnt[:, n_act:NB], lhsT=onesP[:], rhs=pt_d[:], start=True, stop=True)
        cnt = sb.tile([1, NB], F32)
        nc.vector.tensor_copy(cnt[:], ps_cnt[:])
        d_hi = dram.tile([1, NB], F32)
        d_ho = dram.tile([1, NB], F32)
        nc.sync.dma_start(d_hi[:], cnt[:])
        nc.gpsimd.collective_compute(
            "AllReduce", AF.add, replica_groups=[cores],
            ins=[d_hi[:].opt()], outs=[d_ho[:].opt()],
        )
        gcnt = sb.tile([1, NB], F32)
        nc.sync.dma_start(gcnt[:], d_ho[:])
        hist = sb.tile([1, NB], F32)
        nc.vector.tensor_tensor(
            hist[:, 0 : NB - 1], gcnt[:, 0 : NB - 1], gcnt[:, 1:NB], op=AF.subtract
        )
        nc.scalar.copy(hist[:, NB - 1 : NB], gcnt[:, NB - 1 : NB])

    # ---- Phase 6: Otsu on [1, 256] ----
    binc_s = sb.tile([1, NB], F32)
    nc.sync.dma_start(binc_s[:], binc[:])  # arange(256) + 0.5
    centers = sb.tile([1, NB], F32)
    nc.vector.tensor_scalar(centers[:], binc_s[:], span_t[:], None, op0=AF.mult)
    nc.vector.tensor_scalar(centers[:], centers[:], 1.0 / 256.0, None, op0=AF.mult)
    nc.vector.tensor_scalar(centers[:], centers[:], vmin_t[:], None, op0=AF.add)
    hc = sb.tile([1, NB], F32)
    nc.vector.tensor_tensor(hc[:], hist[:], centers[:], op=AF.mult)

    z256 = sb.tile([1, NB], F32)
    nc.vector.memset(z256[:], 0.0)
    w1 = sb.tile([1, NB], F32)
    nc.vector.tensor_tensor_scan(w1[:], hist[:], z256[:], 0.0, op0=AF.add, op1=AF.add)
    c1 = sb.tile([1, NB], F32)
    nc.vector.tensor_tensor_scan(c1[:], hc[:], z256[:], 0.0, op0=AF.add, op1=AF.add)
    w2 = sb.tile([1, NB], F32)
    nc.vector.tensor_tensor_scan(
        w2[:, ::-1], hist[:, ::-1], z256[:], 0.0, op0=AF.add, op1=AF.add
    )
    c2 = sb.tile([1, NB], F32)
    nc.vector.tensor_tensor_scan(
        c2[:, ::-1], hc[:, ::-1], z256[:], 0.0, op0=AF.add, op1=AF.add
    )
    d1 = sb.tile([1, NB], F32)
    nc.vector.tensor_scalar(d1[:], w1[:], 1e-12, None, op0=AF.max)
    nc.vector.reciprocal(d1[:], d1[:])
    m1 = sb.tile([1, NB], F32)
    nc.vector.tensor_tensor(m1[:], c1[:], d1[:], op=AF.mult)
    d2 = sb.tile([1, NB], F32)
    nc.vector.tensor_scalar(d2[:], w2[:], 1e-12, None, op0=AF.max)
    nc.vector.reciprocal(d2[:], d2[:])
    m2 = sb.tile([1, NB], F32)
    nc.vector.tensor_tensor(m2[:], c2[:], d2[:], op=AF.mult)

    nv = NB - 1
    dd = sb.tile([1, nv], F32)
    nc.vector.tensor_tensor(dd[:], m1[:, 0:nv], m2[:, 1:NB], op=AF.subtract)
    ddsq = sb.tile([1, nv], F32)
    nc.vector.tensor_tensor(ddsq[:], dd[:], dd[:], op=AF.mult)
    vv = sb.tile([1, nv], F32)
    nc.vector.tensor_tensor(vv[:], w1[:, 0:nv], w2[:, 1:NB], op=AF.mult)
    var12 = sb.tile([1, nv], F32)
    nc.vector.tensor_tensor(var12[:], vv[:], ddsq[:], op=AF.mult)

    vmx = sb.tile([1, 1], F32)
    nc.vector.tensor_reduce(vmx[:], var12[:], axis=X, op=AF.max)
    eqm = sb.tile([1, nv], F32)
    nc.vector.tensor_scalar(eqm[:], var12[:], vmx[:], None, op0=AF.is_equal)
    BIG = 1.0e9
    # cand = (1 - eqm)*BIG + binc: exact binc (= idx + 0.5) at max positions,
    # ~BIG elsewhere. (1-eqm) computed exactly first to avoid cancellation.
    neq = sb.tile([1, nv], F32)
    nc.vector.tensor_scalar(neq[:], eqm[:], -1.0, None, op0=AF.mult)
    nc.vector.tensor_scalar(neq[:], neq[:], 1.0, None, op0=AF.add)
    cand = sb.tile([1, nv], F32)
    nc.vector.scalar_tensor_tensor(
        cand[:], neq[:], BIG, binc_s[:, 0:nv], op0=AF.mult, op1=AF.add
    )
    idxf = sb.tile([1, 1], F32)
    nc.vector.tensor_reduce(idxf[:], cand[:], axis=X, op=AF.min)
    # t = vmin + ((idx + 0.5) * span) / 256 ; idxf = idx + 0.5 already
    tt = sb.tile([1, 1], F32)
    nc.vector.tensor_scalar(tt[:], idxf[:], span_t[:], None, op0=AF.mult)
    nc.vector.tensor_scalar(tt[:], tt[:], 1.0 / 256.0, None, op0=AF.mult)
    nc.vector.tensor_scalar(tt[:], tt[:], vmin_t[:], None, op0=AF.add)

    # broadcast threshold
    ps_t = ps.tile([P, 1], F32)
    nc.tensor.matmul(ps_t[:], lhsT=ones_1xP[:], rhs=tt[:], start=True, stop=True)
    tB = sb.tile([P, 1], F32)
    nc.vector.tensor_copy(tB[:], ps_t[:])

    if dbg is not None:
        dtile = sb.tile([1, 1024], F32)
        nc.vector.memset(dtile[:], 0.0)
        nc.scalar.copy(dtile[:, 0:1], vmin_t[:])
        nc.scalar.copy(dtile[:, 1:2], span_t[:])
        nc.scalar.copy(dtile[:, 2:3], tt[:])
        nc.scalar.copy(dtile[:, 3:4], idxf[:])
        nc.scalar.copy(dtile[:, 4:5], vmx[:])
        nc.vector.tensor_copy(dtile[:, 256:512], gcnt[:])
        nc.vector.tensor_copy(dtile[:, 512:768], hist[:])
        nc.vector.tensor_copy(dtile[:, 768:1023], var12[:])
        nc.sync.dma_start(dbg[:], dtile[:])

    # ---- Phase 7: binarize + replicate x3 + store ----
    # Replication copies split DVE/ACT so both engines drain the tail.
    with tc.tile_pool(name="outp", bufs=3) as outp:
        for c in range(nchunk):
            gc = gray[:, cpx * c : cpx * (c + 1)]
            b01 = outp.tile([P, cpx], F32, tag="b01")
            nc.vector.tensor_scalar(b01[:], gc, tB[:], None, op0=AF.is_gt)
            o3 = outp.tile([P, 3 * cpx], F32, tag="o3")
            nc.vector.tensor_copy(o3[:, 0 : 3 * cpx : 3], b01[:])
            nc.scalar.copy(o3[:, 1 : 3 * cpx : 3], b01[:])
            nc.scalar.copy(o3[:, 2 : 3 * cpx : 3], b01[:])
            nc.sync.dma_start(y[:, 3 * cpx * c : 3 * cpx * (c + 1)], o3[:])


def build_nc(ppb=FULL_PPB, nchunk=FULL_NCHUNK, debug=False, enable_asserts=False,
             with_dbg=False, hist_mode="hinge", act_frac=0.5515):
    nc = bacc.Bacc(
        "TRN2",
        target_bir_lowering=False,
        debug=debug,
        enable_asserts=enable_asserts,
        num_devices=NCORES,
    )
    x = nc.dram_tensor("x", [P, 3 * ppb], F32, kind="ExternalInput")
    binc = nc.dram_tensor("binc", [1, NB], F32, kind="ExternalInput")
    y = nc.dram_tensor("y", [P, 3 * ppb], F32, kind="ExternalOutput")
    dbg = (
        nc.dram_tensor("dbg", [1, 1024], F32, kind="ExternalOutput")
        if with_dbg
        else None
    )
    with tile.TileContext(nc) as tc:
        with ExitStack() as ctx:
            _kernel_body(
                ctx, tc, x.ap(), binc.ap(), y.ap(), ppb, nchunk,
                dbg=dbg.ap() if dbg is not None else None,
                hist_mode=hist_mode, act_frac=act_frac,
            )
    nc.compile()
    return nc


_NC_CACHE = {}


def _get_nc():
    key = (FULL_PPB, FULL_NCHUNK)
    if key not in _NC_CACHE:
        _NC_CACHE[key] = build_nc()
    return _NC_CACHE[key]


def make_in_maps(inputs_np):
    """inputs_np: [8, 1024, 1536, 3] f32 -> per-core in_maps."""
    binc = (np.arange(NB, dtype=np.float32) + 0.5).reshape(1, NB)
    maps = []
    for c in range(NCORES):
        img = np.ascontiguousarray(inputs_np[c]).reshape(P, 3 * FULL_PPB)
        maps.append({"x": img, "binc": binc})
    return maps


def kernel(inputs: np.ndarray) -> np.ndarray:
    inputs = np.asarray(inputs, dtype=np.float32)
    assert inputs.shape == (8, 1024, 1536, 3), inputs.shape
    nc = _get_nc()
    res = run_bass_kernel_spmd(nc, make_in_maps(inputs), list(range(NCORES)))
    out = np.stack(
        [res.results[c]["y"].reshape(1024, 1536, 3) for c in range(NCORES)], axis=0
    )
    return out


if __name__ == "__main__":
    rng = np.random.default_rng(0)
    x = rng.random((8, 1024, 1536, 3), dtype=np.float32)
    y = kernel(x)
    print(y.shape, y.dtype, y.mean())
